# revision 1
# baseline (speedup 1.0000x reference)
"""NetVLAD (vq_codebook) Trainium2 Bass kernel, 8-way spatially sharded.

Math (verified vs reference to ~2e-7 rel):
  xn = x / ||x||_C per location; logits = conv_w @ xn; soft = softmax_K
  fold(unfold(soft) * top2keep) == soft * cnt, where cnt = 3x3 box-sum of the
  per-cluster top-2 indicator (border discrepancies are killed by the
  (min-dist-to-border)^4 mask). vlad = sa2 @ xn.T - rowsum(sa2) * centroids,
  then intra + global L2 norm.

Sharding: H=192 rows split 8 ways (24 rows/core + 1 halo row each side).
conv/softmax/top2/box-sum local per core; [K,C+1] partial VLAD sums
all-reduced across the 8 cores; final normalization redundantly on each core.
"""
import os
import sys

sys.path.insert(0, "/opt/trn_rl_repo")
os.environ.setdefault("MYCRO_LOCAL_CACHE", "1")

import numpy as np

C, H, W, K = 512, 192, 192, 64
M = 8                      # cores
RPC = H // M               # 24 rows per core
Ls = (RPC + 2) * W         # 4992 slab locations (incl. 1 halo row each side)
NT = Ls // 128             # 39 l-tiles
CT = C // 128              # 4 c-tiles
G = 257                    # odd guard -> v-pass offsets even (bf16 2x mode)
KBW = G + Ls + G           # 5506 keep-buffer width
XW = 8                     # xlc DMA batching (tiles per DMA)

TRACE = False              # set by test.py for profiling runs
_CACHE = {}


def _build_nc():
    import concourse.bass as bass
    import concourse.bass_isa as bass_isa
    import concourse.tile as tile
    from concourse import mybir

    f32 = mybir.dt.float32
    bf16 = mybir.dt.bfloat16
    AF = mybir.ActivationFunctionType
    OP = mybir.AluOpType
    AX = mybir.AxisListType

    nc = bass.Bass()
    xcl = nc.dram_tensor("xcl", [C, Ls], f32, kind="ExternalInput")
    xlcn = nc.dram_tensor("xlcn", [Ls, C + 1], f32, kind="ExternalInput")
    cwt = nc.dram_tensor("cwt", [C, K], f32, kind="ExternalInput")
    cent = nc.dram_tensor("cent", [K, C], f32, kind="ExternalInput")
    sc0 = nc.dram_tensor("sc0", [128, NT], f32, kind="ExternalInput")
    invn = nc.dram_tensor("invn", [128, NT], f32, kind="ExternalInput")
    identb = nc.dram_tensor("identb", [128, 128], bf16, kind="ExternalInput")
    identf = nc.dram_tensor("identf", [128, 128], f32, kind="ExternalInput")
    ones = nc.dram_tensor("ones", [128, 128], f32, kind="ExternalInput")
    y = nc.dram_tensor("y", [K, C + 1], f32, kind="ExternalOutput")

    with tile.TileContext(nc) as tc:
        with tc.tile_pool(name="big", bufs=1) as big:
            # persistent SBUF tensors
            expb = big.tile([128, NT * K], f32, tag="expb")
            tmpb = big.tile([128, NT * K], f32, tag="tmpb")   # also reused as w2
            keeplk = big.tile([128, NT * K], bf16, tag="keeplk")
            kb = big.tile([K, KBW], bf16, tag="kb")
            h3s = big.tile([K, KBW], bf16, tag="h3s")
            cntb = big.tile([K, Ls], bf16, tag="cntb")
            cwt_sb = big.tile([128, CT * K], f32, tag="cwt")
            cent_sb = big.tile([K, C], f32, tag="cent")
            id_sb = big.tile([128, 128], bf16, tag="ident")
            idf_sb = big.tile([128, 128], f32, tag="identf")
            logkl = big.tile([K, Ls], f32, tag="logkl")
            ones_sb = big.tile([128, 128], f32, tag="ones")
            sc_sb = big.tile([128, NT], f32, tag="sc0")
            invn_sb = big.tile([128, NT], f32, tag="invn")
            sume = big.tile([128, NT], f32, tag="sume")
            m1b = big.tile([128, NT], f32, tag="m1b")
            m2b = big.tile([128, NT], f32, tag="m2b")
            isum = big.tile([128, NT], f32, tag="isum")
            scc = big.tile([128, NT], f32, tag="scc")
            vl_sb = big.tile([K, C + 1], f32, tag="vl")
            scr = big.tile([128, 4], f32, tag="scr")

            # input DMAs
            nc.sync.dma_start(id_sb[:], identb[:])
            nc.sync.dma_start(idf_sb[:], identf[:])
            nc.sync.dma_start(ones_sb[:], ones[:])
            nc.sync.dma_start(sc_sb[:], sc0[:])
            nc.sync.dma_start(invn_sb[:], invn[:])
            nc.sync.dma_start(cent_sb[:], cent[:])
            nc.sync.dma_start(
                cwt_sb[:].rearrange("p (t k) -> p t k", k=K),
                cwt[:].rearrange("(t p) k -> p t k", p=128),
            )
            # zero the keep-buffer guards
            nc.vector.memset(kb[:, 0:G], 0.0)
            nc.vector.memset(kb[:, G + Ls:KBW], 0.0)
            # single-wait "touch" ops: each absorbs one DMA completion so no
            # downstream compute instruction needs two sync waits (codegen
            # allows one wait per compute-engine instruction)
            nc.scalar.copy(scr[:, 0:1], invn_sb[:, 0:1])
            nc.vector.tensor_copy(scr[:, 1:2], sc_sb[:, 0:1])

            # One persistent PSUM pool; reuse goes through tag rotation so each
            # PE instruction carries at most one sync wait (codegen limit).
            # Banks: plk 2 + plg 2 + pk 2 + pv0/pv1 2 = 8.
            with tc.tile_pool(name="pp", bufs=1, space="PSUM") as pp:
                pv0 = pp.tile([K, C], f32, tag="pv0", bufs=1)
                pv1 = pp.tile([K, 1], f32, tag="pv1", bufs=1)
                # warm-up burst: absorbs the cwt/ones DMA waits (1-wait codegen
                # limit) and keeps the PE HAM busy so phase 1 runs at 2.4 GHz
                dummy = pp.tile([128, K], f32, tag="plg", bufs=2)
                nc.tensor.matmul(dummy[0:64, 0:64], lhsT=cwt_sb[:, 0:64],
                                 rhs=cwt_sb[:, 0:64], start=True, stop=True)
                for _ in range(12):
                    dummy = pp.tile([128, K], f32, tag="plg", bufs=2)
                    nc.tensor.matmul(dummy[0:64, 0:64], lhsT=ones_sb[:, 0:64],
                                     rhs=ones_sb[:, 0:64], start=True, stop=True)
                # phase 1: logits matmuls + exp (scaled by inv_norm).
                # xcl lives in a scoped pool; its space is reused for the
                # xlcn stream afterwards (fresh addresses -> single-wait DMAs)
                with tc.tile_pool(name="xclp", bufs=1) as xclp:
                    xcl_sb = xclp.tile([128, CT * Ls], f32, tag="xcl")
                    xc3 = xcl[:].rearrange("(ct p) l -> p ct l", p=128)
                    xs3 = xcl_sb[:].rearrange("p (ct l) -> p ct l", l=Ls)
                    NB = Ls // 512          # 9.75 -> use 512-blocks + tail
                    DCH = 4                 # dma chunks (early phase-1 start)
                    csz = Ls // DCH         # 1248 columns per chunk, all c-tiles
                    for j in range(DCH):
                        nc.sync.dma_start(
                            xs3[:, :, j * csz:(j + 1) * csz],
                            xc3[:, :, j * csz:(j + 1) * csz],
                        )
                    # logits blocks in [K, L]: stationary conv_wT (64-col
                    # weight loads), x streams as the moving operand
                    nblk = (Ls + 511) // 512
                    touched = set()
                    for b in range(nblk):
                        w = min(512, Ls - b * 512)
                        for j in range((b * 512) // csz,
                                       (b * 512 + w - 1) // csz + 1):
                            if j not in touched:
                                touched.add(j)
                                dj = pp.tile([128, K], f32, tag="plg", bufs=2)
                                nc.tensor.matmul(
                                    dj[0:64, 0:64],
                                    lhsT=xcl_sb[:, j * csz:j * csz + 64],
                                    rhs=xcl_sb[:, j * csz:j * csz + 64],
                                    start=True, stop=True)
                        plk = pp.tile([K, 512], f32, tag="plk", bufs=2)
                        for ct in range(CT):
                            nc.tensor.matmul(
                                plk[:, 0:w],
                                lhsT=cwt_sb[:, ct * K:(ct + 1) * K],
                                rhs=xcl_sb[:, ct * Ls + b * 512:
                                           ct * Ls + b * 512 + w],
                                start=(ct == 0),
                                stop=(ct == CT - 1),
                            )
                        nc.scalar.copy(logkl[:, b * 512:b * 512 + w],
                                       plk[:, 0:w])
                    # transpose to [L-tile, K] and exp with per-location
                    # inv-norm scale; sumexp accumulates for free
                    for t in range(NT):
                        plg = pp.tile([128, K], f32, tag="plg", bufs=2)
                        nc.tensor.transpose(
                            plg[:], logkl[:, t * 128:(t + 1) * 128],
                            idf_sb[0:K, 0:K])
                        nc.scalar.activation(
                            expb[:, t * K:(t + 1) * K], plg[:], AF.Exp,
                            scale=invn_sb[:, t:t + 1],
                            accum_out=sume[:, t:t + 1],
                        )

                # phase 2: batched top-2 keep over the free axis
                e3 = expb[:].rearrange("p (t k) -> p t k", k=K)
                t3 = tmpb[:].rearrange("p (t k) -> p t k", k=K)
                k3 = keeplk[:].rearrange("p (t k) -> p t k", k=K)
                m1bc = m1b[:][:, :, None].broadcast_to([128, NT, K])
                m2bc = m2b[:][:, :, None].broadcast_to([128, NT, K])
                nc.vector.tensor_reduce(m1b[:], e3, axis=AX.X, op=OP.max)
                nc.vector.tensor_tensor(t3, e3, m1bc, op=OP.is_ge)
                nc.vector.scalar_tensor_tensor(
                    t3, t3, -10.0, e3, op0=OP.mult, op1=OP.add)
                nc.vector.tensor_reduce(m2b[:], t3, axis=AX.X, op=OP.max)
                nc.vector.tensor_tensor(k3, e3, m2bc, op=OP.is_ge)
                nc.vector.reciprocal(isum[:], sume[:])
                nc.vector.tensor_mul(scc[:], sc_sb[:], isum[:])

                # phase 3: transpose keep [L,K] -> [K,L] into guarded buffer
                for t in range(NT):
                    pk = pp.tile([K, 128], bf16, tag="pk", bufs=2)
                    nc.tensor.transpose(
                        pk[:], keeplk[:, t * K:(t + 1) * K], id_sb[:])
                    nc.scalar.copy(kb[:, G + t * 128: G + (t + 1) * 128], pk[:])

                # phase 4: separable 3x3 box-sum along flattened L
                # h3s[j] = kb[j] + kb[j+1] + kb[j+2]  (i.e. h[j+1], shifted)
                nc.vector.tensor_add(
                    h3s[:, 0:KBW - 2], kb[:, 0:KBW - 2], kb[:, 2:KBW])
                nc.vector.tensor_add(
                    h3s[:, 0:KBW - 2], h3s[:, 0:KBW - 2], kb[:, 1:KBW - 1])
                # cnt[l] = h[G+l-192] + h[G+l] + h[G+l+192], h[j] = h3s[j-1]
                nc.vector.tensor_add(
                    cntb[:], h3s[:, G - 193:G - 193 + Ls],
                    h3s[:, G + 191:G + 191 + Ls])
                nc.vector.tensor_add(
                    cntb[:], cntb[:], h3s[:, G - 1:G - 1 + Ls])

                # phase 5: transpose cnt back, fuse w2 = (cntT * scc) * exp
                w2 = tmpb
                for t in range(NT):
                    pc = pp.tile([128, K], bf16, tag="plg", bufs=2)
                    nc.tensor.transpose(
                        pc[:], cntb[:, t * 128:(t + 1) * 128], id_sb[:K, :K])
                    nc.vector.scalar_tensor_tensor(
                        w2[:, t * K:(t + 1) * K], pc[:], scc[:, t:t + 1],
                        expb[:, t * K:(t + 1) * K], op0=OP.mult, op1=OP.mult)

                # absorb the w2 DVE wait before the accumulation chain
                dummy2 = pp.tile([128, K], f32, tag="plg", bufs=2)
                nc.tensor.matmul(dummy2[0:64, 0:64], lhsT=w2[:, 0:64],
                                 rhs=w2[:, 0:64], start=True, stop=True)

                # phase 6: VLAD matmul, accumulate [K, C+1] over all l-tiles.
                # Each xlcn wave gets its own buffer (in space freed by xclp)
                # so stream DMAs carry a single sync wait.
                x3 = xlcn[:].rearrange("(a p) c -> p a c", p=128)
                with tc.tile_pool(name="xlc", bufs=1) as xlcp:
                    for w in range((NT + XW - 1) // XW):
                        n = min(XW, NT - w * XW)
                        xt = xlcp.tile([128, XW * (C + 1)], f32, tag=f"xt{w}")
                        nc.sync.dma_start(
                            xt[:, 0:n * (C + 1)].rearrange(
                                "p (a c) -> p a c", c=C + 1),
                            x3[:, w * XW:w * XW + n, :],
                        )
                        for i in range(n):
                            t = w * XW + i
                            lt = w2[:, t * K:(t + 1) * K]
                            nc.tensor.matmul(
                                pv0[:], lhsT=lt,
                                rhs=xt[:, i * (C + 1):i * (C + 1) + C],
                                start=(t == 0), stop=(t == NT - 1))
                            nc.tensor.matmul(
                                pv1[:], lhsT=lt,
                                rhs=xt[:, i * (C + 1) + C:(i + 1) * (C + 1)],
                                start=(t == 0), stop=(t == NT - 1))

                    # phase 7: write this core's [K, C+1] partial sums;
                    # host sums the 8 partials and applies centroid subtraction
                    # and the two L2 normalizations (0.03% of the FLOPs)
                    nc.scalar.copy(vl_sb[:, 0:C], pv0[:])
                    nc.scalar.copy(vl_sb[:, C:C + 1], pv1[:])
                    nc.sync.dma_start(y[:], vl_sb[:])
    n = _prune_waits(nc)
    return nc


def _prune_waits(nc):
    """Drop semaphore waits that are transitively implied by another wait on
    the same instruction.

    The walrus codegen used here allows at most ONE sync wait per
    instruction.  Tile's sem assignment is not transitively minimal: e.g. a
    consumer waits on both a DMA completion and on a PE tick even though the
    DMA itself already waited on that PE tick.  Per-proc completion is
    in-order (engine FIFOs, per-queue DMA), so "sem S reached v" implies all
    waits of every instruction on S's proc with cumulative tick <= v held.
    We compute that closure and greedily delete implied waits.
    """
    insts = [ins for bb in nc.main_func.blocks for ins in bb.instructions]
    # proc name -> ordered [(cumtick, instr)] and instr -> its waits
    proc_events = {}
    waits_of = {}
    for ins in insts:
        si = getattr(ins, "sync_info", None)
        if si is None:
            continue
        ow = list(si.on_wait or [])
        waits_of[id(ins)] = [(w.ant_name, w.wait_value) for w in ow]
        for u in (si.on_update or []):
            if getattr(u, "update_mode", None) not in ("sem-inc", "sem-add-imm"):
                continue
            lst = proc_events.setdefault(u.ant_name, [])
            prev = lst[-1][0] if lst else 0
            lst.append((prev + (u.update_value or 1), ins))

    # holds[(sem, tick_idx)] -> {sem: max_threshold} computed lazily with
    # memoization over prefix positions; iterate to fixpoint.
    import bisect

    def prefix_index(sem, v):
        lst = proc_events.get(sem)
        if not lst:
            return None
        ticks = [t for t, _ in lst]
        i = bisect.bisect_left(ticks, v)
        return i if i < len(lst) else None

    memo = {}

    def holds(sem, v, depth=0):
        """Thresholds guaranteed held once sem >= v."""
        if depth > 6:
            return {}
        i = prefix_index(sem, v)
        if i is None:
            return {}
        key = (sem, i)
        if key in memo:
            return memo[key]
        memo[key] = {}      # cut cycles conservatively
        out = {}
        # Pool (gpsimd) has multiple cores; don't assume in-order there.
        inorder = not sem.startswith("Pool")
        rng = range(i + 1) if inorder else (i,)
        for j in rng:
            _, ins = proc_events[sem][j]
            for (s2, v2) in waits_of.get(id(ins), []):
                if out.get(s2, 0) < v2:
                    out[s2] = v2
                sub = holds(s2, v2, depth + 1)
                for s3, v3 in sub.items():
                    if out.get(s3, 0) < v3:
                        out[s3] = v3
        memo[key] = out
        return out

    # cumulative tick of each instruction on its own update proc
    own_tick = {}
    for sem, lst in proc_events.items():
        for tick, ins in lst:
            own_tick[(id(ins), sem)] = tick

    pruned = 0
    for ins in insts:
        si = getattr(ins, "sync_info", None)
        if si is None or not si.on_wait or len(si.on_wait) < 2:
            continue
        ow = list(si.on_wait)
        kept = list(ow)
        for w in ow:
            if len(kept) == 1:
                break
            # same-queue FIFO: waiting on earlier completions of the very
            # queue this instruction executes on is vacuous (per-queue
            # serial execution); addresses here are disjoint anyway.
            mine = own_tick.get((id(ins), w.ant_name))
            if mine is not None and w.wait_value <= mine - 1:
                kept.remove(w)
                pruned += 1
                continue
            others = [o for o in kept if o is not w]
            for o in others:
                h = holds(o.ant_name, o.wait_value)
                if h.get(w.ant_name, 0) >= w.wait_value:
                    kept.remove(w)
                    pruned += 1
                    break
        si.on_wait = kept
    return pruned


def _host_prep(x, conv_w, centroids):
    from concourse import mybir
    bf16np = mybir.dt.np(mybir.dt.bfloat16)

    x = np.ascontiguousarray(x, dtype=np.float32)
    L = H * W
    norm = np.sqrt((x.astype(np.float64) ** 2).sum(0))
    norm = np.maximum(norm, 1e-12).astype(np.float32)       # [H,W]
    inv_norm = (1.0 / norm).astype(np.float32)
    ii = np.arange(H, dtype=np.float32)
    mi = np.minimum(H - 1 - ii, ii)
    m = np.minimum(mi[:, None], mi[None, :]).astype(np.float32)
    m2 = m * m
    minv = (m2 * m2) * inv_norm                              # [H,W]

    xpad = np.zeros((C, H + 2, W), np.float32)
    xpad[:, 1:H + 1, :] = x
    # transposed layout with norm column, padded rows
    xtn = np.zeros(((H + 2) * W, C + 1), np.float32)
    xtn[W:(H + 1) * W, 0:C] = x.reshape(C, L).T
    xtn[W:(H + 1) * W, C] = norm.reshape(L)
    invn_pad = np.zeros((H + 2) * W, np.float32)
    invn_pad[W:(H + 1) * W] = inv_norm.reshape(L)
    minv_pad = np.zeros((H + 2) * W, np.float32)
    minv_pad[W:(H + 1) * W] = minv.reshape(L)

    cwt = np.ascontiguousarray(conv_w.T, dtype=np.float32)   # [C,K]
    cent = np.ascontiguousarray(centroids, dtype=np.float32)
    identb = np.eye(128, dtype=np.float32).astype(bf16np)
    identf = np.eye(128, dtype=np.float32)
    ones = np.ones((128, 128), np.float32)

    in_maps = []
    for core in range(M):
        r0 = core * RPC
        sl = slice(r0 * W, (r0 + RPC + 2) * W)               # slab in padded coords
        sc0c = minv_pad[sl].copy()
        sc0c[0:W] = 0.0                                      # halo rows contribute 0
        sc0c[(RPC + 1) * W:] = 0.0
        in_maps.append({
            "xcl": np.ascontiguousarray(
                xpad[:, r0:r0 + RPC + 2, :].reshape(C, Ls)),
            "xlcn": np.ascontiguousarray(xtn[sl]),
            "cwt": cwt,
            "cent": cent,
            "sc0": np.ascontiguousarray(sc0c.reshape(NT, 128).T),
            "invn": np.ascontiguousarray(invn_pad[sl].reshape(NT, 128).T.copy()),
            "identb": identb,
            "identf": identf,
            "ones": ones,
        })
    return in_maps


def _ensure_ntff_hook():
    """Install the axon NTFF profile hook if the image's antenv lacks it."""
    import types
    try:
        from antenv.axon_hooks import get_axon_ntff_profile_hook  # noqa: F401
        return
    except ImportError:
        pass
    if "/root/.axon_site" not in sys.path:
        sys.path.insert(0, "/root/.axon_site")
    from trn_agent_boot.trn_boot import _ntff_profile_via_ctypes
    hook = _ntff_profile_via_ctypes("/opt/axon/libaxon_pjrt.so")
    mod = types.ModuleType("antenv.axon_hooks")
    mod.get_axon_ntff_profile_hook = lambda: hook
    mod.set_axon_ntff_profile_hook = lambda h: None
    import antenv
    antenv.axon_hooks = mod
    sys.modules["antenv.axon_hooks"] = mod


def _install_neff_cache():
    """Cache compiled NEFFs across processes, keyed by BIR content hash."""
    import hashlib
    import shutil
    import concourse.bass2jax as b2j

    orig = b2j.compile_bir_kernel
    if getattr(orig, "_neff_cached", False):
        return

    def cached(bir_json, tmpdir, neff_name="file.neff"):
        h = hashlib.sha256(
            bir_json if isinstance(bir_json, bytes) else bir_json.encode()
        ).hexdigest()[:24]
        cdir = "/tmp/neff_cache"
        os.makedirs(cdir, exist_ok=True)
        cpath = os.path.join(cdir, h + ".neff")
        if os.path.exists(cpath):
            dst = os.path.join(tmpdir, neff_name)
            os.makedirs(tmpdir, exist_ok=True)
            shutil.copy(cpath, dst)
            return dst
        out = orig(bir_json, tmpdir, neff_name=neff_name)
        shutil.copy(out, cpath)
        return out

    cached._neff_cached = True
    b2j.compile_bir_kernel = cached


def kernel(x, conv_w, centroids):
    import concourse.bass_utils as bu
    from concourse.bass_utils import run_bass_kernel_spmd
    _install_neff_cache()
    if TRACE:
        _ensure_ntff_hook()
        bu.upload_artifacts = lambda tmpdir: "local://" + tmpdir

    if "nc" not in _CACHE:
        _CACHE["nc"] = _build_nc()
    nc = _CACHE["nc"]
    in_maps = _host_prep(np.asarray(x), np.asarray(conv_w), np.asarray(centroids))
    res = run_bass_kernel_spmd(nc, in_maps, list(range(M)), trace=TRACE)
    _CACHE["last"] = res
    red = np.zeros((K, C + 1), np.float32)
    for r in res.results:
        red += np.asarray(r["y"], dtype=np.float32)
    vlad = red[:, :C] - red[:, C:C + 1] * np.asarray(centroids, np.float32)
    vlad /= np.maximum(np.sqrt((vlad ** 2).sum(1))[:, None], 1e-12)
    v = vlad.reshape(1, K * C)
    v /= np.maximum(np.sqrt((v ** 2).sum()), 1e-12)
    return v.astype(np.float32)



# revision 9
# speedup vs baseline: 3.1272x; 3.1272x over previous
"""NetVLAD (vq_codebook) Trainium2 Bass kernel, 8-way spatially sharded. v2.

Math (validated vs reference to ~5e-4 rel in numpy):
  xn = x / ||x||_C per location (HOST); logits = cwt.T @ xn per l-tile
  directly in [L,K] layout (stationary x-tile, moving conv weights);
  e = exp(logits); top-2 keep via masked double-max; cnt = 3x3 box-sum of
  keep done as 5 banded shift-matrix matmuls on the PE (no transposes);
  w2 = e * (1/sumexp) * cnt in bf16;
  VLAD partial sums [K, C+1] = w2.T @ [xn*mask4 | mask4] accumulated over
  all l-tiles, all-reduced on host; border mask^4 and 1/norm are folded
  into the host-prepared xlcn stream so no on-chip scaling is needed.

All HBM streams are bf16 (tolerance 2e-2 >> bf16 error ~5e-4): halves DMA
vs f32 and runs PE matmuls at 1 cycle/row instead of 4.

Sharding: H=192 rows split 8 ways (24 rows/core + 1 halo row each side).
Everything local per core; [K,C+1] partials reduced on host.
"""
import os
import sys

sys.path.insert(0, "/opt/trn_rl_repo")
os.environ.setdefault("MYCRO_LOCAL_CACHE", "1")

import numpy as np

C, H, W, K = 512, 192, 192, 64
M = 8                      # cores
RPC = H // M               # 24 rows per core
Ls = (RPC + 2) * W         # 4992 slab locations (incl. 1 halo row each side)
NT = Ls // 128             # 39 l-tiles
CT = C // 128              # 4 c-tiles
CH = 8                     # tiles per pipeline chunk
CHUNKS = [(c, min(c + CH, NT)) for c in range(0, NT, CH)]

TRACE = False              # set by test.py for profiling runs
_CACHE = {}


def _build_nc():
    import concourse.bass as bass
    import concourse.tile as tile
    from concourse import mybir

    f32 = mybir.dt.float32
    bf16 = mybir.dt.bfloat16
    AF = mybir.ActivationFunctionType
    OP = mybir.AluOpType
    AX = mybir.AxisListType

    nc = bass.Bass()
    xcl = nc.dram_tensor("xcl", [C, Ls], bf16, kind="ExternalInput")
    xlcn = nc.dram_tensor("xlcn", [Ls, C + 1], bf16, kind="ExternalInput")
    cwt = nc.dram_tensor("cwt", [C, K], bf16, kind="ExternalInput")
    shm = nc.dram_tensor("shm", [128, 5 * 128], bf16, kind="ExternalInput")
    y = nc.dram_tensor("y", [K, C + 1], f32, kind="ExternalOutput")

    with tile.TileContext(nc) as tc:
        with tc.tile_pool(name="big", bufs=1) as big:
            xcl_sb = big.tile([128, CT * Ls], bf16, tag="xcl")
            xlcn_sb = big.tile([128, NT * (C + 1)], bf16, tag="xlcn")
            cwt_sb = big.tile([128, CT * K], bf16, tag="cwt")
            shm_sb = big.tile([128, 5 * 128], bf16, tag="shm")
            expb = big.tile([128, NT * K], f32, tag="expb")
            tmpb = big.tile([128, NT * K], f32, tag="tmpb")
            eeb = big.tile([128, NT * K], f32, tag="eeb")
            keeplk = big.tile([128, NT * K], bf16, tag="keeplk")
            w2b = big.tile([128, NT * K], bf16, tag="w2b")
            m1b = big.tile([128, NT], f32, tag="m1b")
            m2b = big.tile([128, NT], f32, tag="m2b")
            sume = big.tile([128, NT], f32, tag="sume")
            isume = big.tile([128, NT], f32, tag="isume")
            vl_sb = big.tile([K, C + 1], f32, tag="vl")

            # input DMAs: small first, then xcl chunks, then xlcn waves
            nc.sync.dma_start(
                cwt_sb[:].rearrange("p (t k) -> p t k", k=K),
                cwt[:].rearrange("(t p) k -> p t k", p=128),
            )
            nc.sync.dma_start(shm_sb[:], shm[:])
            xc3 = xcl[:].rearrange("(ct p) l -> p ct l", p=128)
            xs3 = xcl_sb[:].rearrange("p (ct l) -> p ct l", l=Ls)
            for (t0, t1) in CHUNKS:
                nc.sync.dma_start(
                    xs3[:, :, t0 * 128:t1 * 128],
                    xc3[:, :, t0 * 128:t1 * 128],
                )
            xl3d = xlcn[:].rearrange("(a p) c -> p a c", p=128)
            xl3s = xlcn_sb[:].rearrange("p (a c) -> p a c", c=C + 1)
            for (t0, t1) in CHUNKS:
                nc.sync.dma_start(xl3d_dst := xl3s[:, t0:t1, :], xl3d[:, t0:t1, :])

            e3f = expb[:].rearrange("p (t k) -> p t k", k=K)
            t3f = tmpb[:].rearrange("p (t k) -> p t k", k=K)
            ee3f = eeb[:].rearrange("p (t k) -> p t k", k=K)
            k3f = keeplk[:].rearrange("p (t k) -> p t k", k=K)
            w23f = w2b[:].rearrange("p (t k) -> p t k", k=K)

            with tc.tile_pool(name="pp", bufs=1, space="PSUM") as pp:
                pv0 = pp.tile([K, C], f32, tag="pv0", bufs=1)
                pv1 = pp.tile([K, 1], f32, tag="pv1", bufs=1)
                pcnts = {}

                # warm-up burst: ramps the PE HAM clock while first DMA
                # chunks land; first one absorbs the cwt DMA wait, one the
                # shm DMA wait
                for i in range(12):
                    src = shm_sb if i == 1 else cwt_sb
                    dmy = pp.tile([128, K], f32, tag="dmy", bufs=2)
                    nc.tensor.matmul(dmy[0:64, 0:64], lhsT=src[:, 0:64],
                                     rhs=src[:, 0:64], start=True, stop=True)

                def emit_logits(ci):
                    t0, t1 = CHUNKS[ci]
                    # dummy absorbs this chunk's xcl DMA completion so real
                    # matmuls carry only the psum-rotation wait
                    dmy = pp.tile([128, K], f32, tag="dmy", bufs=2)
                    nc.tensor.matmul(
                        dmy[0:64, 0:64],
                        lhsT=xcl_sb[:, t0 * 128:t0 * 128 + 64],
                        rhs=xcl_sb[:, t0 * 128:t0 * 128 + 64],
                        start=True, stop=True)
                    plog = pp.tile([128, CH * K], f32, tag="plog", bufs=2)
                    for t in range(t0, t1):
                        for ct in range(CT):
                            nc.tensor.matmul(
                                plog[:, (t - t0) * K:(t - t0 + 1) * K],
                                lhsT=xcl_sb[:, ct * Ls + t * 128:
                                            ct * Ls + (t + 1) * 128],
                                rhs=cwt_sb[:, ct * K:(ct + 1) * K],
                                start=(ct == 0), stop=(ct == CT - 1))
                    return plog

                def emit_exp(ci, plog):
                    t0, t1 = CHUNKS[ci]
                    for t in range(t0, t1):
                        nc.scalar.activation(
                            expb[:, t * K:(t + 1) * K],
                            plog[:, (t - t0) * K:(t - t0 + 1) * K],
                            AF.Exp, accum_out=sume[:, t:t + 1])

                def emit_dve(ci):
                    t0, t1 = CHUNKS[ci]
                    ch = t1 - t0
                    e3 = e3f[:, t0:t1]
                    t3 = t3f[:, t0:t1]
                    k3 = k3f[:, t0:t1]
                    m1bc = m1b[:, t0:t1][:, :, None].broadcast_to([128, ch, K])
                    m2bc = m2b[:, t0:t1][:, :, None].broadcast_to([128, ch, K])
                    nc.vector.reciprocal(isume[:, t0:t1], sume[:, t0:t1])
                    nc.vector.tensor_reduce(m1b[:, t0:t1], e3, axis=AX.X,
                                            op=OP.max)
                    nc.vector.tensor_tensor(t3, e3, m1bc, op=OP.is_ge)
                    nc.vector.scalar_tensor_tensor(
                        t3, t3, -10.0, e3, op0=OP.mult, op1=OP.add)
                    nc.vector.tensor_reduce(m2b[:, t0:t1], t3, axis=AX.X,
                                            op=OP.max)
                    nc.vector.tensor_tensor(k3, e3, m2bc, op=OP.is_ge)

                def emit_pool_ee(ci):
                    t0, t1 = CHUNKS[ci]
                    ch = t1 - t0
                    ibc = isume[:, t0:t1][:, :, None].broadcast_to([128, ch, K])
                    nc.vector.tensor_tensor(ee3f[:, t0:t1], e3f[:, t0:t1],
                                            ibc, op=OP.mult)

                def emit_cnt(ci):
                    t0, t1 = CHUNKS[ci]
                    pcnt = pp.tile([128, CH * K], f32, tag="pcnt", bufs=2)
                    pcnts[ci] = pcnt
                    jlo = {t: max(-2, -t) for t in range(t0, t1)}
                    jhi = {t: min(2, NT - 1 - t) for t in range(t0, t1)}
                    for j in range(-2, 3):
                        for t in range(t0, t1):
                            if not (jlo[t] <= j <= jhi[t]):
                                continue
                            s = t + j
                            nc.tensor.matmul(
                                pcnt[:, (t - t0) * K:(t - t0 + 1) * K],
                                lhsT=shm_sb[:, (j + 2) * 128:(j + 3) * 128],
                                rhs=keeplk[:, s * K:(s + 1) * K],
                                start=(j == jlo[t]), stop=(j == jhi[t]))

                def emit_pool_w2(ci):
                    # on DVE: GPSIMD cannot read PSUM (pcnt)
                    t0, t1 = CHUNKS[ci]
                    pc3 = pcnts[ci][:, 0:(t1 - t0) * K].rearrange(
                        "p (t k) -> p t k", k=K)
                    nc.vector.tensor_tensor(w23f[:, t0:t1], ee3f[:, t0:t1],
                                            pc3, op=OP.mult)

                def emit_vlad(ci):
                    t0, t1 = CHUNKS[ci]
                    # dummy absorbs the xlcn wave DMA completion
                    dmy = pp.tile([128, K], f32, tag="dmy", bufs=2)
                    nc.tensor.matmul(
                        dmy[0:64, 0:64],
                        lhsT=xlcn_sb[:, t0 * (C + 1):t0 * (C + 1) + 64],
                        rhs=xlcn_sb[:, t0 * (C + 1):t0 * (C + 1) + 64],
                        start=True, stop=True)
                    for t in range(t0, t1):
                        lt = w2b[:, t * K:(t + 1) * K]
                        nc.tensor.matmul(
                            pv0[:], lhsT=lt,
                            rhs=xlcn_sb[:, t * (C + 1):t * (C + 1) + C],
                            start=(t == 0), stop=(t == NT - 1))
                        nc.tensor.matmul(
                            pv1[:], lhsT=lt,
                            rhs=xlcn_sb[:, t * (C + 1) + C:(t + 1) * (C + 1)],
                            start=(t == 0), stop=(t == NT - 1))

                for ci in range(len(CHUNKS)):
                    plog = emit_logits(ci)
                    emit_exp(ci, plog)
                    emit_dve(ci)
                    emit_pool_ee(ci)
                    if ci >= 1:
                        emit_cnt(ci - 1)
                        emit_pool_w2(ci - 1)
                        emit_vlad(ci - 1)
                last = len(CHUNKS) - 1
                emit_cnt(last)
                emit_pool_w2(last)
                emit_vlad(last)

                nc.scalar.copy(vl_sb[:, 0:C], pv0[:])
                nc.scalar.copy(vl_sb[:, C:C + 1], pv1[:])
                nc.sync.dma_start(y[:], vl_sb[:])
    _prune_waits(nc)
    return nc


def _prune_waits(nc):
    """Drop semaphore waits that are transitively implied by another wait on
    the same instruction (see kernel_baseline.py for the full rationale)."""
    insts = [ins for bb in nc.main_func.blocks for ins in bb.instructions]
    proc_events = {}
    waits_of = {}
    carried = {}   # engine -> waits of non-updating instrs (e.g. Ldweights)
    for ins in insts:
        si = getattr(ins, "sync_info", None)
        if si is None:
            continue
        ow = list(si.on_wait or [])
        waits_of[id(ins)] = [(w.ant_name, w.wait_value) for w in ow]
        ups = [u for u in (si.on_update or [])
               if getattr(u, "update_mode", None) in ("sem-inc", "sem-add-imm")]
        eng = getattr(ins, "engine", None)
        if not ups:
            # a waiting-but-not-updating instruction (Ldweights): its waits
            # are guaranteed held once the NEXT updating instruction on the
            # same engine ticks (in-order issue; LDW completes before its MM)
            if ow and eng is not None:
                carried.setdefault(eng, []).extend(waits_of[id(ins)])
            continue
        if eng in carried and carried[eng]:
            waits_of[id(ins)] = waits_of[id(ins)] + carried.pop(eng)
        for u in ups:
            lst = proc_events.setdefault(u.ant_name, [])
            prev = lst[-1][0] if lst else 0
            lst.append((prev + (u.update_value or 1), ins))

    import bisect

    def prefix_index(sem, v):
        lst = proc_events.get(sem)
        if not lst:
            return None
        ticks = [t for t, _ in lst]
        i = bisect.bisect_left(ticks, v)
        return i if i < len(lst) else None

    memo = {}

    def holds(sem, v, depth=0):
        if depth > 6:
            return {}
        i = prefix_index(sem, v)
        if i is None:
            return {}
        key = (sem, i)
        if key in memo:
            return memo[key]
        memo[key] = {}
        out = {}
        inorder = not sem.startswith("Pool")
        rng = range(i + 1) if inorder else (i,)
        for j in rng:
            _, ins = proc_events[sem][j]
            for (s2, v2) in waits_of.get(id(ins), []):
                if out.get(s2, 0) < v2:
                    out[s2] = v2
                sub = holds(s2, v2, depth + 1)
                for s3, v3 in sub.items():
                    if out.get(s3, 0) < v3:
                        out[s3] = v3
        memo[key] = out
        return out

    own_tick = {}
    for sem, lst in proc_events.items():
        for tick, ins in lst:
            own_tick[(id(ins), sem)] = tick

    pruned = 0
    for ins in insts:
        si = getattr(ins, "sync_info", None)
        if si is None or not si.on_wait or len(si.on_wait) < 2:
            continue
        ow = list(si.on_wait)
        kept = list(ow)
        for w in ow:
            if len(kept) == 1:
                break
            mine = own_tick.get((id(ins), w.ant_name))
            if mine is not None and w.wait_value <= mine - 1:
                kept.remove(w)
                pruned += 1
                continue
            others = [o for o in kept if o is not w]
            for o in others:
                h = holds(o.ant_name, o.wait_value)
                if h.get(w.ant_name, 0) >= w.wait_value:
                    kept.remove(w)
                    pruned += 1
                    break
        si.on_wait = kept
    return pruned


def _host_prep(x, conv_w, centroids):
    from concourse import mybir
    bf16np = mybir.dt.np(mybir.dt.bfloat16)

    x = np.ascontiguousarray(x, dtype=np.float32)
    L = H * W
    xf = x.reshape(C, L)
    norm = np.sqrt((xf.astype(np.float64) ** 2).sum(0))
    inv_norm = (1.0 / np.maximum(norm, 1e-12)).astype(np.float32)
    xn = xf * inv_norm[None, :]
    ii = np.arange(H, dtype=np.float32)
    mi = np.minimum(H - 1 - ii, ii)
    m_ = np.minimum(mi[:, None], mi[None, :]).astype(np.float32)
    m2 = m_ * m_
    mask4 = (m2 * m2).reshape(L)

    xn_pad = np.zeros((C, (H + 2) * W), np.float32)
    xn_pad[:, W:(H + 1) * W] = xn
    sc0_pad = np.zeros(((H + 2) * W,), np.float32)
    sc0_pad[W:(H + 1) * W] = mask4

    cwt = np.ascontiguousarray(conv_w.T).astype(bf16np)     # [C, K]

    # banded shift matrices for the 3x3 box-sum over flattened L (W=192)
    delta = np.array([-193, -192, -191, -1, 0, 1, 191, 192, 193])
    q = np.arange(128)
    shm = np.zeros((5, 128, 128), np.float32)               # [j+2, q, i]
    for jj in range(-2, 3):
        for d in delta:
            ivals = q - d + 128 * jj                        # i = q - (d - 128j)
            ok = (ivals >= 0) & (ivals < 128)
            shm[jj + 2, q[ok], ivals[ok]] = 1.0
    shm = np.ascontiguousarray(shm.transpose(1, 0, 2).reshape(128, 5 * 128)
                               ).astype(bf16np)

    in_maps = []
    for core in range(M):
        r0 = core * RPC
        sl = slice(r0 * W, (r0 + RPC + 2) * W)
        sc0c = sc0_pad[sl].copy()
        sc0c[0:W] = 0.0
        sc0c[(RPC + 1) * W:] = 0.0
        xsc = np.empty((Ls, C + 1), np.float32)
        xsc[:, 0:C] = xn_pad[:, sl].T * sc0c[:, None]
        xsc[:, C] = sc0c
        in_maps.append({
            "xcl": np.ascontiguousarray(xn_pad[:, sl]).astype(bf16np),
            "xlcn": xsc.astype(bf16np),
            "cwt": cwt,
            "shm": shm,
        })
    return in_maps


def _ensure_ntff_hook():
    """Install the axon NTFF profile hook if the image's antenv lacks it."""
    import types
    try:
        from antenv.axon_hooks import get_axon_ntff_profile_hook  # noqa: F401
        return
    except ImportError:
        pass
    if "/root/.axon_site" not in sys.path:
        sys.path.insert(0, "/root/.axon_site")
    from trn_agent_boot.trn_boot import _ntff_profile_via_ctypes
    hook = _ntff_profile_via_ctypes("/opt/axon/libaxon_pjrt.so")
    mod = types.ModuleType("antenv.axon_hooks")
    mod.get_axon_ntff_profile_hook = lambda: hook
    mod.set_axon_ntff_profile_hook = lambda h: None
    import antenv
    antenv.axon_hooks = mod
    sys.modules["antenv.axon_hooks"] = mod


def _install_neff_cache():
    """Cache compiled NEFFs across processes, keyed by BIR content hash."""
    import hashlib
    import shutil
    import concourse.bass2jax as b2j

    orig = b2j.compile_bir_kernel
    if getattr(orig, "_neff_cached", False):
        return

    def cached(bir_json, tmpdir, neff_name="file.neff"):
        h = hashlib.sha256(
            bir_json if isinstance(bir_json, bytes) else bir_json.encode()
        ).hexdigest()[:24]
        cdir = "/tmp/neff_cache"
        os.makedirs(cdir, exist_ok=True)
        cpath = os.path.join(cdir, h + ".neff")
        if os.path.exists(cpath):
            dst = os.path.join(tmpdir, neff_name)
            os.makedirs(tmpdir, exist_ok=True)
            shutil.copy(cpath, dst)
            return dst
        out = orig(bir_json, tmpdir, neff_name=neff_name)
        shutil.copy(out, cpath)
        return out

    cached._neff_cached = True
    b2j.compile_bir_kernel = cached


def kernel(x, conv_w, centroids):
    import concourse.bass_utils as bu
    from concourse.bass_utils import run_bass_kernel_spmd
    _install_neff_cache()
    if TRACE:
        _ensure_ntff_hook()
        bu.upload_artifacts = lambda tmpdir: "local://" + tmpdir

    if "nc" not in _CACHE:
        _CACHE["nc"] = _build_nc()
    nc = _CACHE["nc"]
    in_maps = _host_prep(np.asarray(x), np.asarray(conv_w), np.asarray(centroids))
    res = run_bass_kernel_spmd(nc, in_maps, list(range(M)), trace=TRACE)
    _CACHE["last"] = res
    red = np.zeros((K, C + 1), np.float32)
    for r in res.results:
        red += np.asarray(r["y"], dtype=np.float32)
    vlad = red[:, :C] - red[:, C:C + 1] * np.asarray(centroids, np.float32)
    vlad /= np.maximum(np.sqrt((vlad ** 2).sum(1))[:, None], 1e-12)
    v = vlad.reshape(1, K * C)
    v /= np.maximum(np.sqrt((v ** 2).sum()), 1e-12)
    return v.astype(np.float32)
